# revision 4
# baseline (speedup 1.0000x reference)
"""CrossAttention (channel attention) Trainium2 kernel.

Math (per batch element b):
    q = x Wq^T ; k = y Wk^T ; v = y Wv^T          (N=4096 tokens, C=1024 ch)
    per head h (H=16, D=64):
      scores_h = (Qh^T Kh) * D^-0.5 = Wq_h (x^T y) Wk_h^T * s   (D x D)
      attn_h = softmax(scores_h, axis=-1)
      z_h    = Vh attn_h^T                         (N x D)
    out = z Wp^T + bp

Reassociated (saves ~40% FLOPs and avoids transposing x):
    G   = y^T x                    (C x C)   contraction over n: natural layouts
    A   = G^T Wk^T                 (C x C)
    S_h = (s*Wq_h) A_h             (D x D)  -> softmax (unnormalized probs P_h,
                                              row sums r)
    M_h = P_h Wv_h                 (D x C);  Mall[ci, h*D+d] = M_h[d, ci]/r_d
    P   = Mall Wp^T                (C x C)
    out = y P + bp                 (N x C)

Sharding: pure data-parallel over batch B=8 across the 8 NeuronCores.
All on-chip matmuls run in fp16 (full PE rate) with fp32 PSUM accumulation.

Schedule notes (v2):
  - y^T comes pre-transposed from the host (ytall input) instead of an
    on-device strided DMA transpose: fewer descriptors, no HBM contention.
  - y is streamed in 512-col pass-halves; x tiles stream as 2x512-col DMAs.
  - weight DMAs are interleaved into the G loop so they land before use.
  - G PSUM is 4 independent [128,1024] tiles so pass 1 / phase A start as
    soon as individual copies drain.
  - M-phase uses a block-diagonal attnT (both heads of a pair) -> 2
    full-width matmuls per pair instead of 4 half-width ones.
  - P and out share one PSUM pool so the out matmuls overlap P's tail.
  - output is written fp16 (host upcasts); final tiles copy in halves.
"""

import numpy as np
import sys

sys.path.insert(0, "/opt/trn_rl_repo")

import concourse.bass as bass  # noqa: E402
import concourse.mybir as mybir  # noqa: E402
import concourse.tile as tile  # noqa: E402
from concourse import bacc  # noqa: E402
from concourse.masks import make_identity  # noqa: E402

F16 = mybir.dt.float16
F32 = mybir.dt.float32
AX = mybir.AxisListType
AF = mybir.ActivationFunctionType

B, N, C, H = 8, 4096, 1024, 16
D = C // H          # 64
SCALE = D ** -0.5
NT = N // 128       # 32 n-tiles
CT = C // 128       # 8 channel tiles
PAIRS = H // 2      # 8 head pairs


def build_kernel():
    nc = bacc.Bacc("TRN2", target_bir_lowering=False)

    x_d = nc.dram_tensor("x16", [N, C], F16, kind="ExternalInput")
    y_d = nc.dram_tensor("y16", [N, C], F16, kind="ExternalInput")
    yt_d = nc.dram_tensor("yt16", [C, N], F16, kind="ExternalInput")   # y.T
    wqts_d = nc.dram_tensor("wqts", [C, C], F16, kind="ExternalInput")  # (Wq*s).T
    wkt_d = nc.dram_tensor("wkt", [C, C], F16, kind="ExternalInput")    # Wk.T
    wv_d = nc.dram_tensor("wv", [C, C], F16, kind="ExternalInput")      # Wv
    wpt_d = nc.dram_tensor("wpt", [C, C], F16, kind="ExternalInput")    # Wp.T
    bp_d = nc.dram_tensor("bp", [C], F32, kind="ExternalInput")
    out_d = nc.dram_tensor("out", [N, C], F16, kind="ExternalOutput")

    with tile.TileContext(nc) as tc:
        with (
            tc.tile_pool(name="persist", bufs=1) as persist,
            tc.tile_pool(name="stream", bufs=4) as stream,
            tc.tile_pool(name="small", bufs=4) as small,
        ):
            g2 = persist.tile([128, CT, C], F16, name="g2_sb", tag="sc1")
            wqts = persist.tile([128, CT, C], F16, name="wqts_sb")
            wkt = persist.tile([128, CT, C], F16, name="wkt_sb")
            wv = persist.tile([128, CT, C], F16, name="wv_sb")
            wpt = persist.tile([128, CT, C], F16, name="wpt_sb")
            ytall = persist.tile([128, CT, N], F16, name="ytall")
            bias = persist.tile([128, C], F32, name="bias_sb")

            # identity blocks for the probs transposes (gpsimd; overlaps DMA)
            id128 = persist.tile([128, 128], F16, name="id128")
            make_identity(nc, id128)
            # identity block living on partitions 64..127: idhi[64+i, i] = 1
            idhi = persist.tile([128, D], F16, name="idhi")
            nc.gpsimd.memset(idhi, 0.0)
            nc.gpsimd.affine_select(
                out=idhi, in_=idhi,
                compare_op=mybir.AluOpType.not_equal,
                fill=1.0, base=-D, pattern=[[-1, D]], channel_multiplier=1,
            )
            # pre-zeroed block-diagonal attnT slots (off-diag stays 0 forever)
            attn_slots = []
            for i in range(4):
                sl = persist.tile([128, 128], F16, name=f"attnT{i}")
                nc.gpsimd.memset(sl, 0.0)
                attn_slots.append(sl)

            # ================= phase 1+2: G = y^T x =====================
            # cj-half passes: pass p computes G rows cj in [4p, 4p+4).
            # lhsT streams y cols [512p, 512p+512) per tile; rhs streams all
            # of x (re-read in pass 1).  Weight DMAs are drip-fed into the
            # same sync FIFO so they arrive paced with the loop.
            with tc.tile_pool(name="ps_g", bufs=1, space="PSUM") as ps_g_pool:
                for p_half in range(2):
                    ps = [ps_g_pool.tile([128, C], F32, name=f"ps_g{j}",
                                         tag=f"psg{j}") for j in range(4)]
                    for nt in range(NT):
                        rsl = slice(nt * 128, (nt + 1) * 128)
                        ycols = slice(p_half * 512, (p_half + 1) * 512)
                        ystr = stream.tile([128, 512], F16, name="ystr",
                                           tag="ystr", bufs=5)
                        nc.sync.dma_start(ystr, y_d[rsl, ycols])
                        x16 = stream.tile([128, C], F16, name="x16", tag="x16",
                                          bufs=5)
                        nc.sync.dma_start(x16[:, 0:512], x_d[rsl, 0:512])
                        nc.sync.dma_start(x16[:, 512:C], x_d[rsl, 512:C])
                        # drip weight tiles: wkt during pass 0, wqts/wv pass 1
                        if nt % 4 == 2:
                            t = nt // 4
                            if p_half == 0:
                                nc.sync.dma_start(
                                    wkt[:, t, :],
                                    wkt_d[t * 128:(t + 1) * 128, :])
                            else:
                                nc.sync.dma_start(
                                    wqts[:, t, :],
                                    wqts_d[t * 128:(t + 1) * 128, :])
                        elif nt % 4 == 0 and p_half == 1:
                            t = nt // 4
                            nc.sync.dma_start(
                                wv[:, t, :], wv_d[t * 128:(t + 1) * 128, :])
                        for cj4 in range(4):
                            for ch in range(2):
                                nc.tensor.matmul(
                                    ps[cj4][:, ch * 512:(ch + 1) * 512],
                                    lhsT=ystr[:, cj4 * 128:(cj4 + 1) * 128],
                                    rhs=x16[:, ch * 512:(ch + 1) * 512],
                                    start=(nt == 0), stop=(nt == NT - 1),
                                )
                    for cj4 in range(4):
                        cj = p_half * 4 + cj4
                        for ch in range(2):
                            csl = slice(ch * 512, (ch + 1) * 512)
                            nc.vector.tensor_copy(out=g2[:, cj, csl],
                                                  in_=ps[cj4][:, csl])

            # remaining constants: wpt + bias + y^T (all well before use)
            for t in range(CT):
                nc.sync.dma_start(wpt[:, t, :], wpt_d[t * 128:(t + 1) * 128, :])
            bp_ap = bp_d[:]
            nc.sync.dma_start(
                bias,
                bass.AP(tensor=bp_ap.tensor, offset=bp_ap.offset,
                        ap=[[0, 128]] + list(bp_ap.ap)),
            )
            for k in range(CT):
                nc.sync.dma_start(ytall[:, k, :], yt_d[k * 128:(k + 1) * 128, :])

            # ================= phase 3: A = G^T Wk^T ====================
            a_sb = persist.tile([128, CT, C], F16, name="a_sb", tag="sc2")
            with tc.tile_pool(name="ps_a", bufs=2, space="PSUM") as ps_a_pool:
                for ci in range(CT):
                    psa = ps_a_pool.tile([128, C], F32, name="ps_a")
                    for cj in range(CT):
                        for ch in range(2):
                            nc.tensor.matmul(
                                psa[:, ch * 512:(ch + 1) * 512],
                                lhsT=g2[:, cj, ci * 128:(ci + 1) * 128],
                                rhs=wkt[:, cj, ch * 512:(ch + 1) * 512],
                                start=(cj == 0), stop=(cj == CT - 1),
                            )
                    for ch in range(2):
                        csl = slice(ch * 512, (ch + 1) * 512)
                        nc.vector.tensor_copy(out=a_sb[:, ci, csl],
                                              in_=psa[:, csl])

            # ====== phase 4+5: scores -> softmax -> Mall^T ==============
            mallT = persist.tile([128, CT, C], F16, name="mallT", tag="sc1")
            with (
                tc.tile_pool(name="ps_s", bufs=3, space="PSUM") as ps_s_pool,
                tc.tile_pool(name="ps_t", bufs=2, space="PSUM") as ps_t_pool,
                tc.tile_pool(name="ps_m", bufs=2, space="PSUM") as ps_m_pool,
            ):
                for t in range(PAIRS):
                    ps_s = ps_s_pool.tile([128, D], F32, name="ps_s")
                    for h2 in range(2):
                        h = 2 * t + h2
                        hsl = slice(h * D, (h + 1) * D)
                        for ci in range(CT):
                            nc.tensor.matmul(
                                ps_s[h2 * D:(h2 + 1) * D, :],
                                lhsT=wqts[:, ci, hsl],
                                rhs=a_sb[:, ci, hsl],
                                start=(ci == 0), stop=(ci == CT - 1),
                            )
                    mx = small.tile([128, 1], F32, name="mx")
                    nc.vector.reduce_max(out=mx, in_=ps_s, axis=AX.X, negate=True)
                    probs = small.tile([128, D], F16, name="probs")
                    sumex = small.tile([128, 1], F32, name="sumex")
                    nc.scalar.activation(
                        out=probs, in_=ps_s, func=AF.Exp,
                        bias=mx, scale=1.0, accum_out=sumex,
                    )
                    rcp = small.tile([128, 1], F32, name="rcp")
                    nc.vector.reciprocal(out=rcp, in_=sumex)

                    # pad to a full 2KB PSUM bank so consecutive pairs'
                    # transpose groups never share a zero region
                    at_ps = ps_t_pool.tile([128, 1024], F16, name="at_ps")
                    nc.tensor.transpose(at_ps[0:D, 0:D], probs[0:D, :],
                                        id128[0:D, 0:D])
                    nc.tensor.transpose(at_ps[D:128, 0:D], probs[D:128, :],
                                        idhi[D:128, :])
                    # block-diagonal attnT: head a at [0:D,0:D], head b at
                    # [D:,D:]; off-diagonal blocks are pre-zeroed.
                    attnT = attn_slots[t % 4]
                    nc.vector.tensor_copy(out=attnT[0:D, 0:D],
                                          in_=at_ps[0:D, 0:D])
                    nc.vector.tensor_copy(out=attnT[D:128, D:128],
                                          in_=at_ps[D:128, 0:D])

                    for ch in range(2):
                        csl = slice(ch * 512, (ch + 1) * 512)
                        ps_m = ps_m_pool.tile([128, 512], F32, name="ps_m")
                        nc.tensor.matmul(ps_m, lhsT=attnT,
                                         rhs=wv[:, t, csl],
                                         start=True, stop=True)
                        nc.vector.tensor_scalar_mul(
                            out=mallT[:, t, csl], in0=ps_m, scalar1=rcp,
                        )

            # ===== phase 6+7: P = Mall Wp^T ; out = y P + bp ============
            # shared PSUM pool so the first out matmuls overlap P's tail.
            p_sb = persist.tile([128, CT, C], F16, name="p_sb", tag="sc2")
            with tc.tile_pool(name="ps_po", bufs=2, space="PSUM") as ps_po_pool:
                for ci in range(CT):
                    psp = ps_po_pool.tile([128, C], F32, name="ps_p")
                    for cp in range(CT):
                        for ch in range(2):
                            nc.tensor.matmul(
                                psp[:, ch * 512:(ch + 1) * 512],
                                lhsT=mallT[:, cp, ci * 128:(ci + 1) * 128],
                                rhs=wpt[:, cp, ch * 512:(ch + 1) * 512],
                                start=(cp == 0), stop=(cp == CT - 1),
                            )
                    for ch in range(2):
                        csl = slice(ch * 512, (ch + 1) * 512)
                        nc.vector.tensor_copy(out=p_sb[:, ci, csl],
                                              in_=psp[:, csl])

                for nt in range(NT):
                    psf = ps_po_pool.tile([128, C], F32, name="ps_f")
                    for k in range(CT):
                        for ch in range(2):
                            nc.tensor.matmul(
                                psf[:, ch * 512:(ch + 1) * 512],
                                lhsT=ytall[:, k, nt * 128:(nt + 1) * 128],
                                rhs=p_sb[:, k, ch * 512:(ch + 1) * 512],
                                start=(k == 0), stop=(k == CT - 1),
                            )
                    osb = stream.tile([128, C], F16, name="osb", tag="osb",
                                      bufs=3)
                    for ch in range(2):
                        csl = slice(ch * 512, (ch + 1) * 512)
                        nc.vector.tensor_add(out=osb[:, csl], in0=psf[:, csl],
                                             in1=bias[:, csl])
                        nc.sync.dma_start(out_d[nt * 128:(nt + 1) * 128, csl],
                                          osb[:, csl])

    nc.compile()
    return nc


_NC_CACHE = None


def _get_nc():
    global _NC_CACHE
    if _NC_CACHE is None:
        _NC_CACHE = build_kernel()
    return _NC_CACHE


def run(inputs, trace=False, **kw):
    from concourse.bass_utils import run_bass_kernel_spmd

    x = np.asarray(inputs["x"], dtype=np.float32)
    y = np.asarray(inputs["y"], dtype=np.float32)
    Wq = np.asarray(inputs["Wq"], dtype=np.float32)
    Wk = np.asarray(inputs["Wk"], dtype=np.float32)
    Wv = np.asarray(inputs["Wv"], dtype=np.float32)
    Wp = np.asarray(inputs["Wp"], dtype=np.float32)
    bp = np.asarray(inputs["bp"], dtype=np.float32)

    wqts = np.ascontiguousarray((Wq.T * np.float32(SCALE)).astype(np.float16))
    wkt = np.ascontiguousarray(Wk.T.astype(np.float16))
    wv16 = np.ascontiguousarray(Wv.astype(np.float16))
    wpt = np.ascontiguousarray(Wp.T.astype(np.float16))

    x16 = [np.ascontiguousarray(x[b].astype(np.float16)) for b in range(B)]
    y16 = [np.ascontiguousarray(y[b].astype(np.float16)) for b in range(B)]
    yt16 = [np.ascontiguousarray(y16[b].T) for b in range(B)]

    nc = _get_nc()
    in_maps = [
        {
            "x16": x16[b],
            "y16": y16[b],
            "yt16": yt16[b],
            "wqts": wqts,
            "wkt": wkt,
            "wv": wv16,
            "wpt": wpt,
            "bp": bp,
        }
        for b in range(B)
    ]
    res = run_bass_kernel_spmd(nc, in_maps, core_ids=list(range(B)),
                               trace=trace, **kw)
    out = np.stack([res.results[b]["out"].astype(np.float32)
                    for b in range(B)], axis=0)
    return out, res


def kernel(**inputs) -> np.ndarray:
    out, _ = run(inputs)
    return out


if __name__ == "__main__":
    nc = build_kernel()
    print("build ok")


# revision 5
# speedup vs baseline: 1.1470x; 1.1470x over previous
"""CrossAttention (channel attention) Trainium2 kernel.

Math (per batch element b):
    q = x Wq^T ; k = y Wk^T ; v = y Wv^T          (N=4096 tokens, C=1024 ch)
    per head h (H=16, D=64):
      scores_h = (Qh^T Kh) * D^-0.5 = Wq_h (x^T y) Wk_h^T * s   (D x D)
      attn_h = softmax(scores_h, axis=-1)
      z_h    = Vh attn_h^T                         (N x D)
    out = z Wp^T + bp

Reassociated (saves ~40% FLOPs and avoids transposing x):
    G   = y^T x                    (C x C)   contraction over n: natural layouts
    A   = G^T Wk^T                 (C x C)
    S_h = (s*Wq_h) A_h             (D x D)  -> softmax (unnormalized probs P_h,
                                              row sums r)
    M_h = P_h Wv_h                 (D x C);  Mall[ci, h*D+d] = M_h[d, ci]/r_d
    P   = Mall Wp^T                (C x C)
    out = y P + bp                 (N x C)

Sharding: pure data-parallel over batch B=8 across the 8 NeuronCores.
All on-chip matmuls run in fp16 (full PE rate) with fp32 PSUM accumulation.

Schedule notes (v3):
  - y^T comes pre-transposed from the host (yt16) and lands in y16's SBUF
    slot after pass 1; no on-device strided transpose.
  - DMAs use full 2KB-row descriptors; weight tiles drip during pass 1
    (x-only, bandwidth headroom) so everything lands before its phase.
  - G and A share one PSUM pool: A's first matmuls overlap pass-1 drain.
  - softmax phase is stage-major software-pipelined (scores -> max ->
    exp -> transpose -> block-diag attnT -> M) so no engine FIFO
    serializes another pair's chain; psum->SBUF scaled copies run on the
    scalar engine.
  - M-phase uses block-diagonal attnT: 2 full-width matmuls per pair.
  - P and out share one PSUM pool; output is fp16 (host upcasts).
"""

import numpy as np
import sys

sys.path.insert(0, "/opt/trn_rl_repo")

import concourse.bass as bass  # noqa: E402
import concourse.mybir as mybir  # noqa: E402
import concourse.tile as tile  # noqa: E402
from concourse import bacc  # noqa: E402
from concourse.masks import make_identity  # noqa: E402

F16 = mybir.dt.float16
F32 = mybir.dt.float32
AX = mybir.AxisListType
AF = mybir.ActivationFunctionType

B, N, C, H = 8, 4096, 1024, 16
D = C // H          # 64
SCALE = D ** -0.5
NT = N // 128       # 32 n-tiles
CT = C // 128       # 8 channel tiles
PAIRS = H // 2      # 8 head pairs


def build_kernel():
    nc = bacc.Bacc("TRN2", target_bir_lowering=False)

    x_d = nc.dram_tensor("x16", [N, C], F16, kind="ExternalInput")
    y_d = nc.dram_tensor("y16", [N, C], F16, kind="ExternalInput")
    yt_d = nc.dram_tensor("yt16", [C, N], F16, kind="ExternalInput")   # y.T
    wqts_d = nc.dram_tensor("wqts", [C, C], F16, kind="ExternalInput")  # (Wq*s).T
    wkt_d = nc.dram_tensor("wkt", [C, C], F16, kind="ExternalInput")    # Wk.T
    wv_d = nc.dram_tensor("wv", [C, C], F16, kind="ExternalInput")      # Wv
    wpt_d = nc.dram_tensor("wpt", [C, C], F16, kind="ExternalInput")    # Wp.T
    bp_d = nc.dram_tensor("bp", [C], F32, kind="ExternalInput")
    out_d = nc.dram_tensor("out", [N, C], F16, kind="ExternalOutput")

    with tile.TileContext(nc) as tc:
        with (
            tc.tile_pool(name="persist", bufs=1) as persist,
            tc.tile_pool(name="stream", bufs=4) as stream,
        ):
            y16 = persist.tile([128, NT, C], F16, name="y16", tag="ybig")
            g2 = persist.tile([128, CT, C], F16, name="g2_sb", tag="sc1")
            wqts = persist.tile([128, CT, C], F16, name="wqts_sb")
            wkt = persist.tile([128, CT, C], F16, name="wkt_sb")
            wv = persist.tile([128, CT, C], F16, name="wv_sb")
            wpt = persist.tile([128, CT, C], F16, name="wpt_sb")
            bias = persist.tile([128, C], F32, name="bias_sb")

            # identity blocks for the probs transposes (gpsimd; overlaps DMA)
            id128 = persist.tile([128, 128], F16, name="id128")
            make_identity(nc, id128)
            # identity block living on partitions 64..127: idhi[64+i, i] = 1
            idhi = persist.tile([128, D], F16, name="idhi")
            nc.gpsimd.memset(idhi, 0.0)
            nc.gpsimd.affine_select(
                out=idhi, in_=idhi,
                compare_op=mybir.AluOpType.not_equal,
                fill=1.0, base=-D, pattern=[[-1, D]], channel_multiplier=1,
            )
            # pre-zeroed block-diagonal attnT slots (off-diag stays 0 forever)
            attn_slots = []
            for i in range(PAIRS):
                sl = persist.tile([128, 128], F16, name=f"attnT{i}")
                nc.gpsimd.memset(sl, 0.0)
                attn_slots.append(sl)

            # softmax stage buffers (stage-major pipeline)
            scoreb = persist.tile([128, PAIRS, D], F32, name="scoreb")
            probs16 = persist.tile([128, PAIRS, D], F16, name="probs16")
            mxT = persist.tile([128, PAIRS], F32, name="mxT")
            sumex = persist.tile([128, PAIRS], F32, name="sumex")
            rcpT = persist.tile([128, PAIRS], F32, name="rcpT")

            # ========== phases 1-3: G = y^T x ; A = G^T Wk^T ============
            # G runs as two cj-half passes (PSUM holds half of G).  y16 is
            # loaded once (pass 0) and stays resident; x streams per pass.
            # A shares the pool: its tiles reuse G's psum tags, so A's
            # first matmuls run while pass-1 psums drain.
            a_sb = persist.tile([128, CT, C], F16, name="a_sb", tag="sc2")
            with tc.tile_pool(name="ps_ga", bufs=1, space="PSUM") as ps_ga:
                for p_half in range(2):
                    ps = [ps_ga.tile([128, C], F32, name=f"ps_g{j}",
                                     tag=f"psg{j}") for j in range(4)]
                    for nt in range(NT):
                        rsl = slice(nt * 128, (nt + 1) * 128)
                        if p_half == 0:
                            if nt == 0:
                                nc.sync.dma_start(y16[:, 0, 0:512],
                                                  y_d[rsl, 0:512])
                                nc.sync.dma_start(y16[:, 0, 512:C],
                                                  y_d[rsl, 512:C])
                            else:
                                nc.sync.dma_start(y16[:, nt, :], y_d[rsl, :])
                        x16 = stream.tile([128, C], F16, name="x16", tag="x16",
                                          bufs=6)
                        if p_half == 0 and nt == 0:
                            nc.sync.dma_start(x16[:, 0:512], x_d[rsl, 0:512])
                            nc.sync.dma_start(x16[:, 512:C], x_d[rsl, 512:C])
                        else:
                            nc.sync.dma_start(x16, x_d[rsl, :])
                        # drip weight tiles during pass 1 (x-only otherwise)
                        if p_half == 1 and nt % 2 == 0:
                            chunk = nt // 2
                            wtile, wsrc, t = (
                                (wkt, wkt_d, chunk) if chunk < CT
                                else (wqts, wqts_d, chunk - CT)
                            )
                            nc.sync.dma_start(
                                wtile[:, t, :], wsrc[t * 128:(t + 1) * 128, :])
                        for cj4 in range(4):
                            cj = p_half * 4 + cj4
                            for ch in range(2):
                                nc.tensor.matmul(
                                    ps[cj4][:, ch * 512:(ch + 1) * 512],
                                    lhsT=y16[:, nt, cj * 128:(cj + 1) * 128],
                                    rhs=x16[:, ch * 512:(ch + 1) * 512],
                                    start=(nt == 0), stop=(nt == NT - 1),
                                )
                    for cj4 in range(4):
                        cj = p_half * 4 + cj4
                        for ch in range(2):
                            csl = slice(ch * 512, (ch + 1) * 512)
                            nc.vector.tensor_copy(out=g2[:, cj, csl],
                                                  in_=ps[cj4][:, csl])

                # remaining constants (sync ring is free after pass-1 x's;
                # wv lands before phase 5, wpt/bias before 6/7, y^T last)
                for t in range(CT):
                    nc.sync.dma_start(wv[:, t, :],
                                      wv_d[t * 128:(t + 1) * 128, :])
                for t in range(CT):
                    nc.sync.dma_start(wpt[:, t, :],
                                      wpt_d[t * 128:(t + 1) * 128, :])
                bp_ap = bp_d[:]
                nc.sync.dma_start(
                    bias,
                    bass.AP(tensor=bp_ap.tensor, offset=bp_ap.offset,
                            ap=[[0, 128]] + list(bp_ap.ap)),
                )
                ytall = persist.tile([128, CT, N], F16, name="ytall",
                                     tag="ybig")
                for k in range(CT):
                    nc.sync.dma_start(ytall[:, k, :],
                                      yt_d[k * 128:(k + 1) * 128, :])

                # ---- phase 3: A = G^T Wk^T (reuses G's psum tags) ------
                for ci in range(CT):
                    psa = ps_ga.tile([128, C], F32, name="ps_a",
                                     tag=f"psg{ci % 2}")
                    for cj in range(CT):
                        for ch in range(2):
                            nc.tensor.matmul(
                                psa[:, ch * 512:(ch + 1) * 512],
                                lhsT=g2[:, cj, ci * 128:(ci + 1) * 128],
                                rhs=wkt[:, cj, ch * 512:(ch + 1) * 512],
                                start=(cj == 0), stop=(cj == CT - 1),
                            )
                    for ch in range(2):
                        csl = slice(ch * 512, (ch + 1) * 512)
                        nc.vector.tensor_copy(out=a_sb[:, ci, csl],
                                              in_=psa[:, csl])

            # ====== phase 4+5: scores -> softmax -> Mall^T ==============
            # stage-major: each engine's FIFO sees one stage at a time, so
            # no in-order engine serializes another pair's chain.
            mallT = persist.tile([128, CT, C], F16, name="mallT", tag="sc1")
            with (
                tc.tile_pool(name="ps_s", bufs=3, space="PSUM") as ps_s_pool,
                tc.tile_pool(name="ps_t", bufs=2, space="PSUM") as ps_t_pool,
                tc.tile_pool(name="ps_m", bufs=3, space="PSUM") as ps_m_pool,
            ):
                for t in range(PAIRS):      # stage 1: score matmuls
                    ps_s = ps_s_pool.tile([128, 512], F32, name="ps_s")
                    for h2 in range(2):
                        h = 2 * t + h2
                        hsl = slice(h * D, (h + 1) * D)
                        for ci in range(CT):
                            nc.tensor.matmul(
                                ps_s[h2 * D:(h2 + 1) * D, 0:D],
                                lhsT=wqts[:, ci, hsl],
                                rhs=a_sb[:, ci, hsl],
                                start=(ci == 0), stop=(ci == CT - 1),
                            )
                    nc.vector.tensor_copy(out=scoreb[:, t, :],
                                          in_=ps_s[:, 0:D])
                for t in range(PAIRS):      # stage 2a: row maxes
                    nc.vector.reduce_max(out=mxT[:, t:t + 1],
                                         in_=scoreb[:, t, :], axis=AX.X,
                                         negate=True)
                for t in range(PAIRS):      # stage 2b: exp + 1/sum
                    nc.scalar.activation(
                        out=probs16[:, t, :], in_=scoreb[:, t, :], func=AF.Exp,
                        bias=mxT[:, t:t + 1], scale=1.0,
                        accum_out=sumex[:, t:t + 1],
                    )
                    nc.vector.reciprocal(out=rcpT[:, t:t + 1],
                                         in_=sumex[:, t:t + 1])
                for t in range(PAIRS):      # stage 3: transposes -> attnT
                    at_ps = ps_t_pool.tile([128, 1024], F16, name="at_ps")
                    nc.tensor.transpose(at_ps[0:D, 0:D], probs16[0:D, t, :],
                                        id128[0:D, 0:D])
                    nc.tensor.transpose(at_ps[D:128, 0:D], probs16[D:128, t, :],
                                        idhi[D:128, :])
                    attnT = attn_slots[t]
                    nc.vector.tensor_copy(out=attnT[0:D, 0:D],
                                          in_=at_ps[0:D, 0:D])
                    nc.vector.tensor_copy(out=attnT[D:128, D:128],
                                          in_=at_ps[D:128, 0:D])
                for t in range(PAIRS):      # stage 4: M + scaled copy
                    for ch in range(2):
                        csl = slice(ch * 512, (ch + 1) * 512)
                        ps_m = ps_m_pool.tile([128, 512], F32, name="ps_m")
                        nc.tensor.matmul(ps_m, lhsT=attn_slots[t],
                                         rhs=wv[:, t, csl],
                                         start=True, stop=True)
                        nc.scalar.activation(
                            out=mallT[:, t, csl], in_=ps_m, func=AF.Copy,
                            scale=rcpT[:, t:t + 1],
                        )

            # ===== phase 6+7: P = Mall Wp^T ; out = y P + bp ============
            # shared PSUM pool so the first out matmuls overlap P's tail.
            p_sb = persist.tile([128, CT, C], F16, name="p_sb", tag="sc2")
            with tc.tile_pool(name="ps_po", bufs=2, space="PSUM") as ps_po_pool:
                for ci in range(CT):
                    psp = ps_po_pool.tile([128, C], F32, name="ps_p")
                    for cp in range(CT):
                        for ch in range(2):
                            nc.tensor.matmul(
                                psp[:, ch * 512:(ch + 1) * 512],
                                lhsT=mallT[:, cp, ci * 128:(ci + 1) * 128],
                                rhs=wpt[:, cp, ch * 512:(ch + 1) * 512],
                                start=(cp == 0), stop=(cp == CT - 1),
                            )
                    for ch in range(2):
                        csl = slice(ch * 512, (ch + 1) * 512)
                        nc.vector.tensor_copy(out=p_sb[:, ci, csl],
                                              in_=psp[:, csl])

                for nt in range(NT):
                    psf = ps_po_pool.tile([128, C], F32, name="ps_f")
                    for k in range(CT):
                        for ch in range(2):
                            nc.tensor.matmul(
                                psf[:, ch * 512:(ch + 1) * 512],
                                lhsT=ytall[:, k, nt * 128:(nt + 1) * 128],
                                rhs=p_sb[:, k, ch * 512:(ch + 1) * 512],
                                start=(k == 0), stop=(k == CT - 1),
                            )
                    osb = stream.tile([128, C], F16, name="osb", tag="osb",
                                      bufs=3)
                    for ch in range(2):
                        csl = slice(ch * 512, (ch + 1) * 512)
                        nc.vector.tensor_add(out=osb[:, csl], in0=psf[:, csl],
                                             in1=bias[:, csl])
                        nc.sync.dma_start(out_d[nt * 128:(nt + 1) * 128, csl],
                                          osb[:, csl])

    nc.compile()
    return nc


_NC_CACHE = None


def _get_nc():
    global _NC_CACHE
    if _NC_CACHE is None:
        _NC_CACHE = build_kernel()
    return _NC_CACHE


def run(inputs, trace=False, **kw):
    from concourse.bass_utils import run_bass_kernel_spmd

    x = np.asarray(inputs["x"], dtype=np.float32)
    y = np.asarray(inputs["y"], dtype=np.float32)
    Wq = np.asarray(inputs["Wq"], dtype=np.float32)
    Wk = np.asarray(inputs["Wk"], dtype=np.float32)
    Wv = np.asarray(inputs["Wv"], dtype=np.float32)
    Wp = np.asarray(inputs["Wp"], dtype=np.float32)
    bp = np.asarray(inputs["bp"], dtype=np.float32)

    wqts = np.ascontiguousarray((Wq.T * np.float32(SCALE)).astype(np.float16))
    wkt = np.ascontiguousarray(Wk.T.astype(np.float16))
    wv16 = np.ascontiguousarray(Wv.astype(np.float16))
    wpt = np.ascontiguousarray(Wp.T.astype(np.float16))

    x16 = [np.ascontiguousarray(x[b].astype(np.float16)) for b in range(B)]
    y16 = [np.ascontiguousarray(y[b].astype(np.float16)) for b in range(B)]
    yt16 = [np.ascontiguousarray(y16[b].T) for b in range(B)]

    nc = _get_nc()
    in_maps = [
        {
            "x16": x16[b],
            "y16": y16[b],
            "yt16": yt16[b],
            "wqts": wqts,
            "wkt": wkt,
            "wv": wv16,
            "wpt": wpt,
            "bp": bp,
        }
        for b in range(B)
    ]
    res = run_bass_kernel_spmd(nc, in_maps, core_ids=list(range(B)),
                               trace=trace, **kw)
    out = np.stack([res.results[b]["out"].astype(np.float32)
                    for b in range(B)], axis=0)
    return out, res


def kernel(**inputs) -> np.ndarray:
    out, _ = run(inputs)
    return out


if __name__ == "__main__":
    nc = build_kernel()
    print("build ok")
